# revision 1
# baseline (speedup 1.0000x reference)
"""DCTResolution2D forward on 8 TRN2 NeuronCores.

Math: for rate_weights-derived masks, the whole reference collapses to
    out[b, c] = P @ x[b, c] @ Q
with P [133, 128] and Q [128, 133] computed on host from rate_weights
(DCT matrices + adaptive-span masks folded together).

Device kernel (per core, data parallel over 2048/8 = 256 slices):
  stage 1: S = matmul(lhsT=X_s, rhs=P^T)  -> S = (P X_s)^T  [j=128, u=133]
  stage 2: O_top = matmul(lhsT=S[:, :128], rhs=Q)  [128, 133]  (rows 0..127)
           bottom 5 rows of GROUP slices batched: one matmul
           lhsT = BB [128, 5*GROUP], rhs = Q -> [5*GROUP, 133]
"""

import numpy as np

H = W = 128
NEW_H = NEW_W = 133
B, C = 32, 64
N_CORES = 8
NSLICE = (B * C) // N_CORES  # 256 slices per core
GROUP = 16  # slices per DMA group

_SMOOTH = 4.0
_MAX_RATE = 2.0
_MIN_RATE = 0.0
_MIN_SHAPE = 1.0


def _dct_mat(n_):
    n = np.arange(n_)[None, :].astype(np.float64)
    k = np.arange(n_)[:, None].astype(np.float64)
    d = np.cos(np.pi * (2 * n + 1) * k / (2 * n_)) * np.sqrt(2.0 / n_)
    d[0] *= 1.0 / np.sqrt(2.0)
    return d


def _compute_pq(rate_weights):
    rw = np.asarray(rate_weights, np.float64)
    cur = np.array([H, W], np.float64)
    min_allowed = np.maximum(
        (np.array([_MIN_SHAPE, _MIN_SHAPE]) - _SMOOTH) / cur,
        np.array([_MIN_RATE, _MIN_RATE]),
    )
    r = np.clip(rw, min_allowed, np.array([_MAX_RATE, _MAX_RATE]))
    crop = cur * r
    vmask = np.clip((_SMOOTH + crop[0] - np.arange(NEW_H)) / _SMOOTH, 0, 1)
    hmask = np.clip((_SMOOTH + crop[1] - np.arange(NEW_W)) / _SMOOTH, 0, 1)
    dh, dw, dh2, dw2 = _dct_mat(H), _dct_mat(W), _dct_mat(NEW_H), _dct_mat(NEW_W)
    p = (dh2[:H, :].T * vmask[None, :H]) @ dh  # [133, 128]
    q = dw.T @ (hmask[:W, None] * dw2[:W, :])  # [128, 133]
    return p.astype(np.float32), q.astype(np.float32)


def _build_nc(nslice=NSLICE, group=GROUP, passes=1, cfg=None):
    cfg = cfg or {}
    group = cfg.get("group", group)
    b_xin = cfg.get("xin", 3)
    b_mid = cfg.get("mid", 8)
    b_out = cfg.get("out", 3)
    b_ps1 = cfg.get("ps1", 4)
    b_ps2 = cfg.get("ps2", 3)
    bf16x2 = cfg.get("bf16x2", False)
    xf16 = cfg.get("xf16", False)  # x and P^T shipped/multiplied as fp16
    sf16 = cfg.get("sf16", False)  # stage-2 (S^T @ Q) in fp16
    pair = cfg.get("pair", False)  # 2 slices per PSUM bank, wide copies
    mode = cfg.get("mode", "full")  # full | dma | compute
    import concourse.bass as bass
    import concourse.tile as tile
    from concourse import bacc, mybir

    f32 = mybir.dt.float32
    bf16 = mybir.dt.bfloat16
    nc = bacc.Bacc("TRN2", target_bir_lowering=False, debug=False)

    # x is host-pre-permuted to [H, nslice, W] so each partition's DMA run
    # is contiguous; otop likewise [H, nslice, NEW_W]. In bf16x2 mode the
    # host ships x pre-split as hi/lo bf16 arrays (xh + xl == x to ~16
    # mantissa bits) and P^T likewise, so stage 1 runs as three 1-cycle/row
    # bf16 matmuls accumulated in PSUM instead of one 4-cycle/row fp32.
    if bf16x2:
        xh = nc.dram_tensor("xh", [H, nslice, W], bf16, kind="ExternalInput").ap()
        xl = nc.dram_tensor("xl", [H, nslice, W], bf16, kind="ExternalInput").ap()
        pth = nc.dram_tensor("pth", [H, NEW_H], bf16, kind="ExternalInput").ap()
        ptl = nc.dram_tensor("ptl", [H, NEW_H], bf16, kind="ExternalInput").ap()
    else:
        xdt = mybir.dt.float16 if xf16 else f32
        x = nc.dram_tensor("x", [H, nslice, W], xdt, kind="ExternalInput").ap()
        pt = nc.dram_tensor("pt", [H, NEW_H], xdt, kind="ExternalInput").ap()
    sdt = mybir.dt.float16 if sf16 else f32
    q = nc.dram_tensor("q", [W, NEW_W], sdt, kind="ExternalInput").ap()
    otop = nc.dram_tensor("otop", [H, nslice, NEW_W], f32, kind="ExternalOutput").ap()
    obot = nc.dram_tensor(
        "obot", [nslice, NEW_H - H, NEW_W], f32, kind="ExternalOutput"
    ).ap()

    nbot = NEW_H - H  # 5
    with tile.TileContext(nc) as tc:
        with (
            tc.tile_pool(name="const", bufs=1) as cpool,
            tc.tile_pool(name="xin", bufs=b_xin) as xpool,
            tc.tile_pool(name="mid", bufs=b_mid) as mpool,
            tc.tile_pool(name="bot", bufs=2) as bpool,
            tc.tile_pool(name="out", bufs=b_out) as opool,
            tc.tile_pool(name="ps1", bufs=b_ps1, space="PSUM") as ps1,
            tc.tile_pool(name="ps2", bufs=b_ps2, space="PSUM") as ps2,
            tc.tile_pool(name="ps3", bufs=cfg.get("ps3", 1), space="PSUM") as ps3,
        ):
            if bf16x2:
                pth_sb = cpool.tile([H, NEW_H], bf16)
                nc.sync.dma_start(pth_sb[:], pth[:])
                ptl_sb = cpool.tile([H, NEW_H], bf16)
                nc.sync.dma_start(ptl_sb[:], ptl[:])
            else:
                pt_sb = cpool.tile([H, NEW_H], xdt)
                nc.sync.dma_start(pt_sb[:], pt[:])
            q_sb = cpool.tile([W, NEW_W], sdt)
            nc.sync.dma_start(q_sb[:], q[:])

            for g in [gg for _ in range(passes) for gg in range(nslice // group)]:
                sl = slice(g * group, (g + 1) * group)
                if bf16x2:
                    xht = xpool.tile([H, group, W], bf16, tag="xh")
                    nc.sync.dma_start(xht[:], xh[:, sl, :])
                    xlt = xpool.tile([H, group, W], bf16, tag="xl")
                    nc.sync.dma_start(xlt[:], xl[:, sl, :])
                else:
                    xt = xpool.tile([H, group, W], xdt)
                    if mode != "compute":
                        nc.sync.dma_start(xt[:], x[:, sl, :])
                    else:
                        nc.gpsimd.memset(xt[:, 0, :1], 0.0)
                ot = opool.tile([H, group, NEW_W], f32)
                bsub = min(16, group)
                nsub = group // bsub
                ob_sbs = []
                for sub in range(nsub):
                    bb = bpool.tile([W, bsub * nbot], sdt, tag="bb")
                    if pair and mode != "dma":
                        for kk in range(0, bsub, 2):
                            k = sub * bsub + kk
                            s_ps = ps1.tile([W, 2, NEW_H], f32)
                            nc.tensor.matmul(s_ps[:, 0, :], xt[:, k, :], pt_sb[:])
                            nc.tensor.matmul(s_ps[:, 1, :], xt[:, k + 1, :], pt_sb[:])
                            s_sb = mpool.tile([W, 2, H], sdt)
                            nc.scalar.copy(s_sb[:], s_ps[:, :, 0:H])
                            nc.vector.tensor_copy(
                                bb[:, kk * nbot : (kk + 2) * nbot].rearrange(
                                    "p (n r) -> p n r", n=2
                                ),
                                s_ps[:, :, H:NEW_H],
                            )
                            o_ps = ps2.tile([H, 2, NEW_W], f32)
                            nc.tensor.matmul(o_ps[:, 0, :], s_sb[:, 0, :], q_sb[:])
                            nc.tensor.matmul(o_ps[:, 1, :], s_sb[:, 1, :], q_sb[:])
                            nc.vector.tensor_copy(ot[:, k : k + 2, :], o_ps[:])
                    for kk in range(bsub if (mode != "dma" and not pair) else 0):
                        k = sub * bsub + kk
                        s_ps = ps1.tile([W, NEW_H], f32)
                        if bf16x2:
                            nc.tensor.matmul(
                                s_ps[:], xht[:, k, :], pth_sb[:], start=True, stop=False
                            )
                            nc.tensor.matmul(
                                s_ps[:], xht[:, k, :], ptl_sb[:], start=False, stop=False
                            )
                            nc.tensor.matmul(
                                s_ps[:], xlt[:, k, :], pth_sb[:], start=False, stop=True
                            )
                        else:
                            nc.tensor.matmul(s_ps[:], xt[:, k, :], pt_sb[:])
                        s_sb = mpool.tile([W, H], sdt)
                        nc.scalar.copy(s_sb[:], s_ps[:, 0:H])
                        nc.vector.tensor_copy(
                            bb[:, kk * nbot : (kk + 1) * nbot], s_ps[:, H:NEW_H]
                        )
                        o_ps = ps2.tile([H, NEW_W], f32)
                        nc.tensor.matmul(o_ps[:], s_sb[:], q_sb[:])
                        nc.vector.tensor_copy(ot[:, k, :], o_ps[:])
                    ob_sb = bpool.tile([bsub * nbot, NEW_W], f32, tag="ob")
                    ob_sbs.append(ob_sb)
                    if mode != "dma":
                        ob_ps = ps3.tile([bsub * nbot, NEW_W], f32)
                        nc.tensor.matmul(ob_ps[:], bb[:], q_sb[:])
                        nc.vector.tensor_copy(ob_sb[:], ob_ps[:])
                    else:
                        nc.gpsimd.memset(ob_sb[:, :1], 0.0)
                if mode == "dma":
                    nc.gpsimd.memset(ot[:, 0, :1], 0.0)
                if mode != "compute":
                    nc.sync.dma_start(otop[:, sl, :], ot[:])
                    for sub in range(nsub):
                        ssub = slice(
                            g * group + sub * bsub, g * group + (sub + 1) * bsub
                        )
                        nc.sync.dma_start(
                            obot[ssub].rearrange("n r v -> (n r) v"), ob_sbs[sub][:]
                        )

    nc.compile()
    return nc


_CACHE = {}

# Best HW-measured config: 2-slices-per-PSUM-bank compute with wide
# PSUM->SBUF copies, 32-slice DMA groups, everything fp32 (rel err ~4e-7).
CFG = {"pair": True, "group": 32}
MAP_KW = {"bf16x2": False, "xf16": False}


def _get_nc():
    if "nc" not in _CACHE:
        _CACHE["nc"] = _build_nc(cfg=CFG)
    return _CACHE["nc"]


def make_in_maps(x, rate_weights, bf16x2=False, xf16=False):
    p, q = _compute_pq(rate_weights)
    pt = np.ascontiguousarray(p.T)  # [128, 133]
    q = np.ascontiguousarray(q)
    xs = np.asarray(x, np.float32).reshape(N_CORES, NSLICE, H, W)
    # per-core permute to [H, NSLICE, W] so device DMA runs are contiguous
    shards = np.ascontiguousarray(xs.transpose(0, 2, 1, 3))
    if bf16x2:
        import ml_dtypes

        bf = ml_dtypes.bfloat16
        xh = shards.astype(bf)
        xl = (shards - xh.astype(np.float32)).astype(bf)
        pth = pt.astype(bf)
        ptl = (pt - pth.astype(np.float32)).astype(bf)
        return [
            {"xh": xh[c], "xl": xl[c], "pth": pth, "ptl": ptl, "q": q}
            for c in range(N_CORES)
        ]
    if xf16:
        shards = shards.astype(np.float16)
        pt = pt.astype(np.float16)
    return [{"x": shards[c], "pt": pt, "q": q} for c in range(N_CORES)]


def run(x, rate_weights, trace=False):
    """Returns (full_output, BassKernelResults)."""
    from concourse import bass_utils

    in_maps = make_in_maps(x, rate_weights, **MAP_KW)
    nc = _get_nc()
    res = bass_utils.run_bass_kernel_spmd(
        nc, in_maps, core_ids=list(range(N_CORES)), trace=trace
    )
    out = np.empty((B * C, NEW_H, NEW_W), np.float32)
    for c in range(N_CORES):
        r = res.results[c]
        lo, hi = c * NSLICE, (c + 1) * NSLICE
        out[lo:hi, :H, :] = r["otop"].transpose(1, 0, 2)
        out[lo:hi, H:, :] = r["obot"]
    return out.reshape(B, C, NEW_H, NEW_W), res


def kernel(x, rate_weights):
    out, _ = run(x, rate_weights)
    return out



# revision 9
# speedup vs baseline: 32.8759x; 32.8759x over previous
"""DCTResolution2D forward on 8 TRN2 NeuronCores.

Math: for rate_weights-derived masks, the whole reference collapses to
    out[b, c] = P @ x[b, c] @ Q
with P [133, 128] and Q [128, 133] computed on host from rate_weights
(DCT matrices + adaptive-span masks folded together).

Device kernel (per core, data parallel over 2048/8 = 256 slices):
  stage 1: S = matmul(lhsT=X_s, rhs=P^T)  -> S = (P X_s)^T  [j=128, u=133]
  stage 2: O_top = matmul(lhsT=S[:, :128], rhs=Q)  [128, 133]  (rows 0..127)
           bottom 5 rows of GROUP slices batched: one matmul
           lhsT = BB [128, 5*GROUP], rhs = Q -> [5*GROUP, 133]
"""

import numpy as np

H = W = 128
NEW_H = NEW_W = 133
B, C = 32, 64
N_CORES = 8
NSLICE = (B * C) // N_CORES  # 256 slices per core
GROUP = 16  # slices per DMA group

_SMOOTH = 4.0
_MAX_RATE = 2.0
_MIN_RATE = 0.0
_MIN_SHAPE = 1.0


def _dct_mat(n_):
    n = np.arange(n_)[None, :].astype(np.float64)
    k = np.arange(n_)[:, None].astype(np.float64)
    d = np.cos(np.pi * (2 * n + 1) * k / (2 * n_)) * np.sqrt(2.0 / n_)
    d[0] *= 1.0 / np.sqrt(2.0)
    return d


def _compute_pq(rate_weights):
    rw = np.asarray(rate_weights, np.float64)
    cur = np.array([H, W], np.float64)
    min_allowed = np.maximum(
        (np.array([_MIN_SHAPE, _MIN_SHAPE]) - _SMOOTH) / cur,
        np.array([_MIN_RATE, _MIN_RATE]),
    )
    r = np.clip(rw, min_allowed, np.array([_MAX_RATE, _MAX_RATE]))
    crop = cur * r
    vmask = np.clip((_SMOOTH + crop[0] - np.arange(NEW_H)) / _SMOOTH, 0, 1)
    hmask = np.clip((_SMOOTH + crop[1] - np.arange(NEW_W)) / _SMOOTH, 0, 1)
    dh, dw, dh2, dw2 = _dct_mat(H), _dct_mat(W), _dct_mat(NEW_H), _dct_mat(NEW_W)
    p = (dh2[:H, :].T * vmask[None, :H]) @ dh  # [133, 128]
    q = dw.T @ (hmask[:W, None] * dw2[:W, :])  # [128, 133]
    return p.astype(np.float32), q.astype(np.float32)


def _build_nc(nslice=NSLICE, group=GROUP, passes=1, cfg=None):
    cfg = cfg or {}
    group = cfg.get("group", group)
    b_xin = cfg.get("xin", 3)
    b_mid = cfg.get("mid", 8)
    b_out = cfg.get("out", 3)
    b_ps1 = cfg.get("ps1", 4)
    b_ps2 = cfg.get("ps2", 3)
    bf16x2 = cfg.get("bf16x2", False)
    xf16 = cfg.get("xf16", False)  # x and P^T shipped/multiplied as fp16
    sf16 = cfg.get("sf16", False)  # stage-2 (S^T @ Q) in fp16
    of16 = cfg.get("of16", False)  # outputs written/DMAed as fp16
    pair = cfg.get("pair", False)  # 2 slices per PSUM bank, wide copies
    mode = cfg.get("mode", "full")  # full | dma | compute
    import concourse.bass as bass
    import concourse.tile as tile
    from concourse import bacc, mybir

    f32 = mybir.dt.float32
    bf16 = mybir.dt.bfloat16
    nc = bacc.Bacc("TRN2", target_bir_lowering=False, debug=False)

    # x is host-pre-permuted to [H, nslice, W] so each partition's DMA run
    # is contiguous; otop likewise [H, nslice, NEW_W]. In bf16x2 mode the
    # host ships x pre-split as hi/lo bf16 arrays (xh + xl == x to ~16
    # mantissa bits) and P^T likewise, so stage 1 runs as three 1-cycle/row
    # bf16 matmuls accumulated in PSUM instead of one 4-cycle/row fp32.
    if bf16x2:
        xh = nc.dram_tensor("xh", [H, nslice, W], bf16, kind="ExternalInput").ap()
        xl = nc.dram_tensor("xl", [H, nslice, W], bf16, kind="ExternalInput").ap()
        pth = nc.dram_tensor("pth", [H, NEW_H], bf16, kind="ExternalInput").ap()
        ptl = nc.dram_tensor("ptl", [H, NEW_H], bf16, kind="ExternalInput").ap()
    else:
        xdt = mybir.dt.float16 if xf16 else f32
        x = nc.dram_tensor("x", [H, nslice, W], xdt, kind="ExternalInput").ap()
        pt = nc.dram_tensor("pt", [H, NEW_H], xdt, kind="ExternalInput").ap()
    sdt = mybir.dt.float16 if sf16 else f32
    odt = mybir.dt.float16 if of16 else f32
    q = nc.dram_tensor("q", [W, NEW_W], sdt, kind="ExternalInput").ap()
    otop = nc.dram_tensor("otop", [H, nslice, NEW_W], odt, kind="ExternalOutput").ap()
    obot = nc.dram_tensor(
        "obot", [nslice, NEW_H - H, NEW_W], odt, kind="ExternalOutput"
    ).ap()

    nbot = NEW_H - H  # 5
    with tile.TileContext(nc) as tc:
        with (
            tc.tile_pool(name="const", bufs=1) as cpool,
            tc.tile_pool(name="xin", bufs=b_xin) as xpool,
            tc.tile_pool(name="mid", bufs=b_mid) as mpool,
            tc.tile_pool(name="bot", bufs=2) as bpool,
            tc.tile_pool(name="out", bufs=b_out) as opool,
            tc.tile_pool(name="ps1", bufs=b_ps1, space="PSUM") as ps1,
            tc.tile_pool(name="ps2", bufs=b_ps2, space="PSUM") as ps2,
            tc.tile_pool(name="ps3", bufs=cfg.get("ps3", 1), space="PSUM") as ps3,
        ):
            if bf16x2:
                pth_sb = cpool.tile([H, NEW_H], bf16)
                nc.sync.dma_start(pth_sb[:], pth[:])
                ptl_sb = cpool.tile([H, NEW_H], bf16)
                nc.sync.dma_start(ptl_sb[:], ptl[:])
            else:
                pt_sb = cpool.tile([H, NEW_H], xdt)
                nc.sync.dma_start(pt_sb[:], pt[:])
            q_sb = cpool.tile([W, NEW_W], sdt)
            nc.sync.dma_start(q_sb[:], q[:])

            for g in [gg for _ in range(passes) for gg in range(nslice // group)]:
                sl = slice(g * group, (g + 1) * group)
                if bf16x2:
                    xht = xpool.tile([H, group, W], bf16, tag="xh")
                    nc.sync.dma_start(xht[:], xh[:, sl, :])
                    xlt = xpool.tile([H, group, W], bf16, tag="xl")
                    nc.sync.dma_start(xlt[:], xl[:, sl, :])
                else:
                    xt = xpool.tile([H, group, W], xdt)
                    if mode != "compute":
                        nc.sync.dma_start(xt[:], x[:, sl, :])
                    else:
                        nc.gpsimd.memset(xt[:, 0, :1], 0.0)
                ot = opool.tile([H, group, NEW_W], odt)
                bsub = min(16, group)
                nsub = group // bsub
                ob_sbs = []
                for sub in range(nsub):
                    bb = bpool.tile([W, bsub * nbot], sdt, tag="bb")
                    if pair and mode != "dma":
                        for kk in range(0, bsub, 2):
                            k = sub * bsub + kk
                            s_ps = ps1.tile([W, 2, NEW_H], f32)
                            nc.tensor.matmul(s_ps[:, 0, :], xt[:, k, :], pt_sb[:])
                            nc.tensor.matmul(s_ps[:, 1, :], xt[:, k + 1, :], pt_sb[:])
                            s_sb = mpool.tile([W, 2, H], sdt)
                            nc.scalar.copy(s_sb[:], s_ps[:, :, 0:H])
                            nc.vector.tensor_copy(
                                bb[:, kk * nbot : (kk + 2) * nbot].rearrange(
                                    "p (n r) -> p n r", n=2
                                ),
                                s_ps[:, :, H:NEW_H],
                            )
                            o_ps = ps2.tile([H, 2, NEW_W], f32)
                            nc.tensor.matmul(o_ps[:, 0, :], s_sb[:, 0, :], q_sb[:])
                            nc.tensor.matmul(o_ps[:, 1, :], s_sb[:, 1, :], q_sb[:])
                            nc.vector.tensor_copy(ot[:, k : k + 2, :], o_ps[:])
                    for kk in range(bsub if (mode != "dma" and not pair) else 0):
                        k = sub * bsub + kk
                        s_ps = ps1.tile([W, NEW_H], f32)
                        if bf16x2:
                            nc.tensor.matmul(
                                s_ps[:], xht[:, k, :], pth_sb[:], start=True, stop=False
                            )
                            nc.tensor.matmul(
                                s_ps[:], xht[:, k, :], ptl_sb[:], start=False, stop=False
                            )
                            nc.tensor.matmul(
                                s_ps[:], xlt[:, k, :], pth_sb[:], start=False, stop=True
                            )
                        else:
                            nc.tensor.matmul(s_ps[:], xt[:, k, :], pt_sb[:])
                        s_sb = mpool.tile([W, H], sdt)
                        nc.scalar.copy(s_sb[:], s_ps[:, 0:H])
                        nc.vector.tensor_copy(
                            bb[:, kk * nbot : (kk + 1) * nbot], s_ps[:, H:NEW_H]
                        )
                        o_ps = ps2.tile([H, NEW_W], f32)
                        nc.tensor.matmul(o_ps[:], s_sb[:], q_sb[:])
                        nc.vector.tensor_copy(ot[:, k, :], o_ps[:])
                    ob_sb = bpool.tile([bsub * nbot, NEW_W], odt, tag="ob")
                    ob_sbs.append(ob_sb)
                    if mode != "dma":
                        ob_ps = ps3.tile([bsub * nbot, NEW_W], f32)
                        nc.tensor.matmul(ob_ps[:], bb[:], q_sb[:])
                        nc.vector.tensor_copy(ob_sb[:], ob_ps[:])
                    else:
                        nc.gpsimd.memset(ob_sb[:, :1], 0.0)
                if mode == "dma":
                    nc.gpsimd.memset(ot[:, 0, :1], 0.0)
                if mode != "compute":
                    nc.sync.dma_start(otop[:, sl, :], ot[:])
                    for sub in range(nsub):
                        ssub = slice(
                            g * group + sub * bsub, g * group + (sub + 1) * bsub
                        )
                        nc.sync.dma_start(
                            obot[ssub].rearrange("n r v -> (n r) v"), ob_sbs[sub][:]
                        )

    nc.compile()
    return nc


def _build_nc_v3(nslice=NSLICE, passes=1, cfg=None):
    """fp16 pipeline; 4-slice stage-1 PSUM tiles (exactly one bank), 3-slice
    stage-2 tiles, bottom rows via extra 5-col matmuls reusing loaded weights.

    Per slice: S128 = (P1 x)^T via matmul(lhsT=x_k, rhs=P1^T) [128 cols],
    strip = (P2 x)^T via matmul(lhsT=x_k, rhs=P2^T) [5 cols, same weights],
    out_top = S128^T @ Q via matmul(lhsT=s_sb, rhs=Q).
    ACT copies S + bottom accumulators (PSUM->SBUF fp16); DVE copies the
    out tiles and bottom outputs. All HBM I/O in fp16.
    """
    cfg = dict(cfg or {})
    group = cfg.get("group", 32)
    sstep = cfg.get("sstep", 4)  # slices per stage-1 PSUM tile (4*128*4B = 1 bank)
    ostep = cfg.get("ostep", 3)  # slices per stage-2 PSUM tile (3*133*4B < 1 bank)
    b_xin = cfg.get("xin", 3)
    b_mid = cfg.get("mid", 3)
    b_out = cfg.get("out", 3)
    b_ps1 = cfg.get("ps1", 3)
    b_ps2 = cfg.get("ps2", 2)
    obat = cfg.get("obat", 4)  # 16-slice subs per obot DMA
    ob_act = cfg.get("ob_act", True)  # bottom-output copy on ACT (else DVE)
    import concourse.tile as tile
    from concourse import bacc, mybir

    f32 = mybir.dt.float32
    f16 = mybir.dt.float16
    nc = bacc.Bacc("TRN2", target_bir_lowering=False, debug=False)

    SUB = 16
    nbot = NEW_H - H  # 5
    assert group % SUB == 0 and nslice % group == 0
    assert (nslice // SUB) % obat == 0
    x = nc.dram_tensor("x", [H, nslice, W], f16, kind="ExternalInput").ap()
    pt1 = nc.dram_tensor("pt1", [H, H], f16, kind="ExternalInput").ap()
    pt2 = nc.dram_tensor("pt2", [H, nbot], f16, kind="ExternalInput").ap()
    q = nc.dram_tensor("q", [W, NEW_W], f16, kind="ExternalInput").ap()
    otop = nc.dram_tensor("otop", [H, nslice, NEW_W], f16, kind="ExternalOutput").ap()
    obot = nc.dram_tensor(
        "obot", [nslice // (SUB * obat), SUB * nbot, obat, NEW_W], f16,
        kind="ExternalOutput",
    ).ap()

    with tile.TileContext(nc) as tc:
        with (
            tc.tile_pool(name="const", bufs=1) as cpool,
            tc.tile_pool(name="xin", bufs=b_xin) as xpool,
            tc.tile_pool(name="mid", bufs=b_mid) as mpool,
            tc.tile_pool(name="bbs", bufs=2) as bbpool,
            tc.tile_pool(name="oba", bufs=2) as obapool,
            tc.tile_pool(name="out", bufs=b_out) as opool,
            tc.tile_pool(name="ps1", bufs=b_ps1, space="PSUM") as ps1,
            tc.tile_pool(name="ps2", bufs=b_ps2, space="PSUM") as ps2,
            tc.tile_pool(name="psb", bufs=2, space="PSUM") as psb,
            tc.tile_pool(name="pso", bufs=1, space="PSUM") as pso,
        ):
            pt1_sb = cpool.tile([H, H], f16)
            nc.sync.dma_start(pt1_sb[:], pt1[:])
            pt2_sb = cpool.tile([H, nbot], f16)
            nc.sync.dma_start(pt2_sb[:], pt2[:])
            q_sb = cpool.tile([W, NEW_W], f16)
            nc.sync.dma_start(q_sb[:], q[:])

            for p in range(passes):
                s_tiles = {}  # tile index (k // sstep within group) -> s_sb
                bb_ps = None
                ob_acc = None
                for g in range(nslice // group):
                    sl = slice(g * group, (g + 1) * group)
                    xt = xpool.tile([H, group, W], f16)
                    nc.sync.dma_start(xt[:], x[:, sl, :])
                    ot = opool.tile([H, group, NEW_W], f16)
                    s2_done = 0  # slices of this group already through stage 2

                    def flush_stage2(upto):
                        nonlocal s2_done
                        while s2_done < upto:
                            m = min(ostep, upto - s2_done)
                            t = s2_done
                            o_ps = ps2.tile([H, ostep, NEW_W], f32)
                            for j in range(m):
                                kk = t + j
                                nc.tensor.matmul(
                                    o_ps[:, j, :],
                                    s_tiles[kk // sstep][:, kk % sstep, :],
                                    q_sb[:],
                                )
                            nc.vector.tensor_copy(
                                ot[:, t : t + m, :], o_ps[:, :m, :]
                            )
                            s2_done += m

                    for k in range(group):
                        gk = g * group + k
                        if k % sstep == 0:
                            s_ps = ps1.tile([W, sstep, H], f32)
                        if gk % SUB == 0:
                            bb_ps = psb.tile([W, SUB, nbot], f32, tag="bb")
                        nc.tensor.matmul(s_ps[:, k % sstep, :], xt[:, k, :], pt1_sb[:])
                        nc.tensor.matmul(
                            bb_ps[:, gk % SUB, :], xt[:, k, :], pt2_sb[:]
                        )
                        if k % sstep == sstep - 1:
                            s_sb = mpool.tile([W, sstep, H], f16)
                            nc.scalar.copy(s_sb[:], s_ps[:])
                            s_tiles[k // sstep] = s_sb
                            # run stage 2 for every full ostep chunk now covered
                            flush_stage2((k + 1) - ((k + 1) % ostep))
                        if gk % SUB == SUB - 1:
                            si = gk // SUB  # global sub index
                            bb_sb = bbpool.tile([W, SUB * nbot], f16)
                            nc.scalar.copy(
                                bb_sb[:].rearrange("p (n r) -> p n r", n=SUB),
                                bb_ps[:],
                            )
                            ob_ps = pso.tile([SUB * nbot, NEW_W], f32)
                            nc.tensor.matmul(ob_ps[:], bb_sb[:], q_sb[:])
                            if si % obat == 0:
                                ob_acc = obapool.tile(
                                    [SUB * nbot, obat, NEW_W], f16, tag="oba"
                                )
                            cp = nc.scalar.copy if ob_act else nc.vector.tensor_copy
                            cp(ob_acc[:, si % obat, :], ob_ps[:])
                            if (si + 1) % obat == 0:
                                nc.sync.dma_start(
                                    obot[(si % (nslice // SUB)) // obat], ob_acc[:]
                                )
                    flush_stage2(group)
                    nc.sync.dma_start(otop[:, sl, :], ot[:])

    nc.compile()
    return nc


_CACHE = {}

# Best HW-measured config: 2-slices-per-PSUM-bank compute with wide
# PSUM->SBUF copies, 32-slice DMA groups, everything fp32 (rel err ~4e-7).
CFG = {"v3": True}
MAP_KW = {"v3": True}


def _get_nc():
    if "nc" not in _CACHE:
        builder = _build_nc_v3 if CFG.get("v3") else _build_nc
        _CACHE["nc"] = builder(cfg=CFG)
    return _CACHE["nc"]


def make_in_maps(x, rate_weights, bf16x2=False, xf16=False, v3=False):
    p, q = _compute_pq(rate_weights)
    pt = np.ascontiguousarray(p.T)  # [128, 133]
    q = np.ascontiguousarray(q)
    xs = np.asarray(x, np.float32).reshape(N_CORES, NSLICE, H, W)
    # per-core permute to [H, NSLICE, W] so device DMA runs are contiguous
    shards = np.ascontiguousarray(xs.transpose(0, 2, 1, 3))
    if v3:
        shards = shards.astype(np.float16)
        pt1 = np.ascontiguousarray(pt[:, :H]).astype(np.float16)
        pt2 = np.ascontiguousarray(pt[:, H:]).astype(np.float16)
        q16 = q.astype(np.float16)
        return [
            {"x": shards[c], "pt1": pt1, "pt2": pt2, "q": q16}
            for c in range(N_CORES)
        ]
    if bf16x2:
        import ml_dtypes

        bf = ml_dtypes.bfloat16
        xh = shards.astype(bf)
        xl = (shards - xh.astype(np.float32)).astype(bf)
        pth = pt.astype(bf)
        ptl = (pt - pth.astype(np.float32)).astype(bf)
        return [
            {"xh": xh[c], "xl": xl[c], "pth": pth, "ptl": ptl, "q": q}
            for c in range(N_CORES)
        ]
    if xf16:
        shards = shards.astype(np.float16)
        pt = pt.astype(np.float16)
    return [{"x": shards[c], "pt": pt, "q": q} for c in range(N_CORES)]


def run(x, rate_weights, trace=False):
    """Returns (full_output, BassKernelResults)."""
    from concourse import bass_utils

    in_maps = make_in_maps(x, rate_weights, **MAP_KW)
    nc = _get_nc()
    res = bass_utils.run_bass_kernel_spmd(
        nc, in_maps, core_ids=list(range(N_CORES)), trace=trace
    )
    out = np.empty((B * C, NEW_H, NEW_W), np.float32)
    nbot = NEW_H - H
    for c in range(N_CORES):
        r = res.results[c]
        lo, hi = c * NSLICE, (c + 1) * NSLICE
        out[lo:hi, :H, :] = r["otop"].transpose(1, 0, 2)
        if CFG.get("v3"):
            ob = r["obot"]  # [nsg, 16*nbot, obat, NEW_W]
            nsg, _, obat, _ = ob.shape
            ob = ob.reshape(nsg, 16, nbot, obat, NEW_W).transpose(0, 3, 1, 2, 4)
            out[lo:hi, H:, :] = ob.reshape(NSLICE, nbot, NEW_W)
        else:
            out[lo:hi, H:, :] = r["obot"]
    return out.reshape(B, C, NEW_H, NEW_W), res


def kernel(x, rate_weights):
    out, _ = run(x, rate_weights)
    return out



# revision 18
# speedup vs baseline: 40.7509x; 1.2395x over previous
"""DCTResolution2D forward on 8 TRN2 NeuronCores.

Math: for rate_weights-derived masks, the whole reference collapses to
    out[b, c] = P @ x[b, c] @ Q
with P [133, 128] and Q [128, 133] computed on host from rate_weights
(DCT matrices + adaptive-span masks folded together).

Active kernel (_build_nc_v3, CFG={"v3": True}): full fp16 I/O pipeline,
data parallel over 2048/8 = 256 slices per core. Per slice k:
  stage 1: S128 = matmul(lhsT=x_k, rhs=P1^T) -> (P1 x_k)^T  [128, 128]
           strip = matmul(lhsT=x_k, rhs=P2^T) -> (P2 x_k)^T [128, 5]
           (P1 = P[:128], P2 = P[128:]; strip reuses the loaded weights)
  stage 2: O_top = matmul(lhsT=S128, rhs=Q) [128, 133]
           bottom: per 16 slices, one matmul lhsT=BB [128, 80], rhs=Q.
ACT copies S tiles + bottom accumulators PSUM->SBUF (fp16 cast); DVE
copies output tiles. All HBM traffic fp16 (~17.4 MB/core/pass); the
kernel sits at the per-core HBM bandwidth roofline (~358 GB/s).
The host casts the fp16 outputs back to float32 while gathering.

The older fp32/mixed builder (_build_nc) is kept for comparison runs.
"""

import numpy as np

H = W = 128
NEW_H = NEW_W = 133
B, C = 32, 64
N_CORES = 8
NSLICE = (B * C) // N_CORES  # 256 slices per core
GROUP = 16  # slices per DMA group

_SMOOTH = 4.0
_MAX_RATE = 2.0
_MIN_RATE = 0.0
_MIN_SHAPE = 1.0


def _dct_mat(n_):
    n = np.arange(n_)[None, :].astype(np.float64)
    k = np.arange(n_)[:, None].astype(np.float64)
    d = np.cos(np.pi * (2 * n + 1) * k / (2 * n_)) * np.sqrt(2.0 / n_)
    d[0] *= 1.0 / np.sqrt(2.0)
    return d


def _compute_pq(rate_weights):
    rw = np.asarray(rate_weights, np.float64)
    cur = np.array([H, W], np.float64)
    min_allowed = np.maximum(
        (np.array([_MIN_SHAPE, _MIN_SHAPE]) - _SMOOTH) / cur,
        np.array([_MIN_RATE, _MIN_RATE]),
    )
    r = np.clip(rw, min_allowed, np.array([_MAX_RATE, _MAX_RATE]))
    crop = cur * r
    vmask = np.clip((_SMOOTH + crop[0] - np.arange(NEW_H)) / _SMOOTH, 0, 1)
    hmask = np.clip((_SMOOTH + crop[1] - np.arange(NEW_W)) / _SMOOTH, 0, 1)
    dh, dw, dh2, dw2 = _dct_mat(H), _dct_mat(W), _dct_mat(NEW_H), _dct_mat(NEW_W)
    p = (dh2[:H, :].T * vmask[None, :H]) @ dh  # [133, 128]
    q = dw.T @ (hmask[:W, None] * dw2[:W, :])  # [128, 133]
    return p.astype(np.float32), q.astype(np.float32)


def _build_nc(nslice=NSLICE, group=GROUP, passes=1, cfg=None):
    cfg = cfg or {}
    group = cfg.get("group", group)
    b_xin = cfg.get("xin", 3)
    b_mid = cfg.get("mid", 8)
    b_out = cfg.get("out", 3)
    b_ps1 = cfg.get("ps1", 4)
    b_ps2 = cfg.get("ps2", 3)
    bf16x2 = cfg.get("bf16x2", False)
    xf16 = cfg.get("xf16", False)  # x and P^T shipped/multiplied as fp16
    sf16 = cfg.get("sf16", False)  # stage-2 (S^T @ Q) in fp16
    of16 = cfg.get("of16", False)  # outputs written/DMAed as fp16
    pair = cfg.get("pair", False)  # 2 slices per PSUM bank, wide copies
    mode = cfg.get("mode", "full")  # full | dma | compute
    import concourse.bass as bass
    import concourse.tile as tile
    from concourse import bacc, mybir

    f32 = mybir.dt.float32
    bf16 = mybir.dt.bfloat16
    nc = bacc.Bacc("TRN2", target_bir_lowering=False, debug=False)

    # x is host-pre-permuted to [H, nslice, W] so each partition's DMA run
    # is contiguous; otop likewise [H, nslice, NEW_W]. In bf16x2 mode the
    # host ships x pre-split as hi/lo bf16 arrays (xh + xl == x to ~16
    # mantissa bits) and P^T likewise, so stage 1 runs as three 1-cycle/row
    # bf16 matmuls accumulated in PSUM instead of one 4-cycle/row fp32.
    if bf16x2:
        xh = nc.dram_tensor("xh", [H, nslice, W], bf16, kind="ExternalInput").ap()
        xl = nc.dram_tensor("xl", [H, nslice, W], bf16, kind="ExternalInput").ap()
        pth = nc.dram_tensor("pth", [H, NEW_H], bf16, kind="ExternalInput").ap()
        ptl = nc.dram_tensor("ptl", [H, NEW_H], bf16, kind="ExternalInput").ap()
    else:
        xdt = mybir.dt.float16 if xf16 else f32
        x = nc.dram_tensor("x", [H, nslice, W], xdt, kind="ExternalInput").ap()
        pt = nc.dram_tensor("pt", [H, NEW_H], xdt, kind="ExternalInput").ap()
    sdt = mybir.dt.float16 if sf16 else f32
    odt = mybir.dt.float16 if of16 else f32
    q = nc.dram_tensor("q", [W, NEW_W], sdt, kind="ExternalInput").ap()
    otop = nc.dram_tensor("otop", [H, nslice, NEW_W], odt, kind="ExternalOutput").ap()
    obot = nc.dram_tensor(
        "obot", [nslice, NEW_H - H, NEW_W], odt, kind="ExternalOutput"
    ).ap()

    nbot = NEW_H - H  # 5
    with tile.TileContext(nc) as tc:
        with (
            tc.tile_pool(name="const", bufs=1) as cpool,
            tc.tile_pool(name="xin", bufs=b_xin) as xpool,
            tc.tile_pool(name="mid", bufs=b_mid) as mpool,
            tc.tile_pool(name="bot", bufs=2) as bpool,
            tc.tile_pool(name="out", bufs=b_out) as opool,
            tc.tile_pool(name="ps1", bufs=b_ps1, space="PSUM") as ps1,
            tc.tile_pool(name="ps2", bufs=b_ps2, space="PSUM") as ps2,
            tc.tile_pool(name="ps3", bufs=cfg.get("ps3", 1), space="PSUM") as ps3,
        ):
            if bf16x2:
                pth_sb = cpool.tile([H, NEW_H], bf16)
                nc.sync.dma_start(pth_sb[:], pth[:])
                ptl_sb = cpool.tile([H, NEW_H], bf16)
                nc.sync.dma_start(ptl_sb[:], ptl[:])
            else:
                pt_sb = cpool.tile([H, NEW_H], xdt)
                nc.sync.dma_start(pt_sb[:], pt[:])
            q_sb = cpool.tile([W, NEW_W], sdt)
            nc.sync.dma_start(q_sb[:], q[:])

            for g in [gg for _ in range(passes) for gg in range(nslice // group)]:
                sl = slice(g * group, (g + 1) * group)
                if bf16x2:
                    xht = xpool.tile([H, group, W], bf16, tag="xh")
                    nc.sync.dma_start(xht[:], xh[:, sl, :])
                    xlt = xpool.tile([H, group, W], bf16, tag="xl")
                    nc.sync.dma_start(xlt[:], xl[:, sl, :])
                else:
                    xt = xpool.tile([H, group, W], xdt)
                    if mode != "compute":
                        nc.sync.dma_start(xt[:], x[:, sl, :])
                    else:
                        nc.gpsimd.memset(xt[:, 0, :1], 0.0)
                ot = opool.tile([H, group, NEW_W], odt)
                bsub = min(16, group)
                nsub = group // bsub
                ob_sbs = []
                for sub in range(nsub):
                    bb = bpool.tile([W, bsub * nbot], sdt, tag="bb")
                    if pair and mode != "dma":
                        for kk in range(0, bsub, 2):
                            k = sub * bsub + kk
                            s_ps = ps1.tile([W, 2, NEW_H], f32)
                            nc.tensor.matmul(s_ps[:, 0, :], xt[:, k, :], pt_sb[:])
                            nc.tensor.matmul(s_ps[:, 1, :], xt[:, k + 1, :], pt_sb[:])
                            s_sb = mpool.tile([W, 2, H], sdt)
                            nc.scalar.copy(s_sb[:], s_ps[:, :, 0:H])
                            nc.vector.tensor_copy(
                                bb[:, kk * nbot : (kk + 2) * nbot].rearrange(
                                    "p (n r) -> p n r", n=2
                                ),
                                s_ps[:, :, H:NEW_H],
                            )
                            o_ps = ps2.tile([H, 2, NEW_W], f32)
                            nc.tensor.matmul(o_ps[:, 0, :], s_sb[:, 0, :], q_sb[:])
                            nc.tensor.matmul(o_ps[:, 1, :], s_sb[:, 1, :], q_sb[:])
                            nc.vector.tensor_copy(ot[:, k : k + 2, :], o_ps[:])
                    for kk in range(bsub if (mode != "dma" and not pair) else 0):
                        k = sub * bsub + kk
                        s_ps = ps1.tile([W, NEW_H], f32)
                        if bf16x2:
                            nc.tensor.matmul(
                                s_ps[:], xht[:, k, :], pth_sb[:], start=True, stop=False
                            )
                            nc.tensor.matmul(
                                s_ps[:], xht[:, k, :], ptl_sb[:], start=False, stop=False
                            )
                            nc.tensor.matmul(
                                s_ps[:], xlt[:, k, :], pth_sb[:], start=False, stop=True
                            )
                        else:
                            nc.tensor.matmul(s_ps[:], xt[:, k, :], pt_sb[:])
                        s_sb = mpool.tile([W, H], sdt)
                        nc.scalar.copy(s_sb[:], s_ps[:, 0:H])
                        nc.vector.tensor_copy(
                            bb[:, kk * nbot : (kk + 1) * nbot], s_ps[:, H:NEW_H]
                        )
                        o_ps = ps2.tile([H, NEW_W], f32)
                        nc.tensor.matmul(o_ps[:], s_sb[:], q_sb[:])
                        nc.vector.tensor_copy(ot[:, k, :], o_ps[:])
                    ob_sb = bpool.tile([bsub * nbot, NEW_W], odt, tag="ob")
                    ob_sbs.append(ob_sb)
                    if mode != "dma":
                        ob_ps = ps3.tile([bsub * nbot, NEW_W], f32)
                        nc.tensor.matmul(ob_ps[:], bb[:], q_sb[:])
                        nc.vector.tensor_copy(ob_sb[:], ob_ps[:])
                    else:
                        nc.gpsimd.memset(ob_sb[:, :1], 0.0)
                if mode == "dma":
                    nc.gpsimd.memset(ot[:, 0, :1], 0.0)
                if mode != "compute":
                    nc.sync.dma_start(otop[:, sl, :], ot[:])
                    for sub in range(nsub):
                        ssub = slice(
                            g * group + sub * bsub, g * group + (sub + 1) * bsub
                        )
                        nc.sync.dma_start(
                            obot[ssub].rearrange("n r v -> (n r) v"), ob_sbs[sub][:]
                        )

    nc.compile()
    return nc


def _build_nc_v3(nslice=NSLICE, passes=1, cfg=None):
    """fp16 pipeline; 4-slice stage-1 PSUM tiles (exactly one bank), 3-slice
    stage-2 tiles, bottom rows via extra 5-col matmuls reusing loaded weights.

    Per slice: S128 = (P1 x)^T via matmul(lhsT=x_k, rhs=P1^T) [128 cols],
    strip = (P2 x)^T via matmul(lhsT=x_k, rhs=P2^T) [5 cols, same weights],
    out_top = S128^T @ Q via matmul(lhsT=s_sb, rhs=Q).
    ACT copies S + bottom accumulators (PSUM->SBUF fp16); DVE copies the
    out tiles and bottom outputs. All HBM I/O in fp16.
    """
    cfg = dict(cfg or {})
    group = cfg.get("group", 32)
    sstep = cfg.get("sstep", 4)  # slices per stage-1 PSUM tile (4*128*4B = 1 bank)
    ostep = cfg.get("ostep", 3)  # slices per stage-2 PSUM tile (3*133*4B < 1 bank)
    b_xin = cfg.get("xin", 3)
    b_mid = cfg.get("mid", 3)
    b_out = cfg.get("out", 3)
    b_ps1 = cfg.get("ps1", 3)
    b_ps2 = cfg.get("ps2", 2)
    obat = cfg.get("obat", 4)  # 16-slice subs per obot DMA
    ob_act = cfg.get("ob_act", True)  # bottom-output copy on ACT (else DVE)
    mode = cfg.get("mode", "full")  # full | dma (I/O only) | compute (no I/O)
    import concourse.tile as tile
    from concourse import bacc, mybir

    f32 = mybir.dt.float32
    f16 = mybir.dt.float16
    nc = bacc.Bacc("TRN2", target_bir_lowering=False, debug=False)

    SUB = 16
    nbot = NEW_H - H  # 5
    assert group % SUB == 0 and nslice % group == 0
    assert (nslice // SUB) % obat == 0
    # dma-mode ships x padded to NEW_W cols so the out-DMAs (sourced from xt)
    # keep full-size contiguous runs; real traffic is within 2% of mode=full
    xw = NEW_W if mode == "dma" else W
    x = nc.dram_tensor("x", [H, nslice, xw], f16, kind="ExternalInput").ap()
    pt1 = nc.dram_tensor("pt1", [H, H], f16, kind="ExternalInput").ap()
    pt2 = nc.dram_tensor("pt2", [H, nbot], f16, kind="ExternalInput").ap()
    q = nc.dram_tensor("q", [W, NEW_W], f16, kind="ExternalInput").ap()
    otop = nc.dram_tensor("otop", [H, nslice, NEW_W], f16, kind="ExternalOutput").ap()
    obot = nc.dram_tensor(
        "obot", [nslice // (SUB * obat), SUB * nbot, obat, NEW_W], f16,
        kind="ExternalOutput",
    ).ap()

    with tile.TileContext(nc) as tc:
        with (
            tc.tile_pool(name="const", bufs=1) as cpool,
            tc.tile_pool(name="xin", bufs=b_xin) as xpool,
            tc.tile_pool(name="mid", bufs=b_mid) as mpool,
            tc.tile_pool(name="bbs", bufs=2) as bbpool,
            tc.tile_pool(name="oba", bufs=2) as obapool,
            tc.tile_pool(name="out", bufs=b_out) as opool,
            tc.tile_pool(name="ps1", bufs=b_ps1, space="PSUM") as ps1,
            tc.tile_pool(name="ps2", bufs=b_ps2, space="PSUM") as ps2,
            tc.tile_pool(name="psb", bufs=2, space="PSUM") as psb,
            tc.tile_pool(name="pso", bufs=1, space="PSUM") as pso,
        ):
            pt1_sb = cpool.tile([H, H], f16)
            nc.sync.dma_start(pt1_sb[:], pt1[:])
            pt2_sb = cpool.tile([H, nbot], f16)
            nc.sync.dma_start(pt2_sb[:], pt2[:])
            q_sb = cpool.tile([W, NEW_W], f16)
            nc.sync.dma_start(q_sb[:], q[:])

            if mode == "dma":
                # I/O-only A/B variant: same DMA byte counts and RAW
                # dependency shape (out waits on in), no compute.
                for p in range(passes):
                    for g in range(nslice // group):
                        sl = slice(g * group, (g + 1) * group)
                        xt = xpool.tile([H, group, NEW_W], f16)
                        nc.sync.dma_start(xt[:], x[:, sl, :])
                        nc.sync.dma_start(otop[:, sl, :], xt[:])
                        xf = xt[:].rearrange("p g w -> p (g w)")
                        if g % 2 == 0:
                            j = (g // 2) % (nslice // (16 * obat))
                            nc.sync.dma_start(
                                obot[j],
                                xf[: 16 * nbot, : obat * NEW_W].rearrange(
                                    "p (a v) -> p a v", a=obat
                                ),
                            )
                nc.compile()
                return nc

            if mode == "compute":
                xt_c = cpool.tile([H, group, W], f16)
                nc.gpsimd.memset(xt_c[:, 0, :1], 0.0)

            for p in range(passes):
                s_tiles = {}  # tile index (k // sstep within group) -> s_sb
                bb_ps = None
                ob_acc = None
                for g in range(nslice // group):
                    sl = slice(g * group, (g + 1) * group)
                    if mode == "compute":
                        xt = xt_c
                    else:
                        xt = xpool.tile([H, group, W], f16)
                        nc.sync.dma_start(xt[:], x[:, sl, :])
                    ot = opool.tile([H, group, NEW_W], f16)
                    s2_done = 0  # slices of this group already through stage 2

                    def flush_stage2(upto):
                        nonlocal s2_done
                        while s2_done < upto:
                            m = min(ostep, upto - s2_done)
                            t = s2_done
                            o_ps = ps2.tile([H, ostep, NEW_W], f32)
                            for j in range(m):
                                kk = t + j
                                nc.tensor.matmul(
                                    o_ps[:, j, :],
                                    s_tiles[kk // sstep][:, kk % sstep, :],
                                    q_sb[:],
                                )
                            nc.vector.tensor_copy(
                                ot[:, t : t + m, :], o_ps[:, :m, :]
                            )
                            s2_done += m

                    for k in range(group):
                        gk = g * group + k
                        if k % sstep == 0:
                            s_ps = ps1.tile([W, sstep, H], f32)
                        if gk % SUB == 0:
                            bb_ps = psb.tile([W, SUB, nbot], f32, tag="bb")
                        nc.tensor.matmul(s_ps[:, k % sstep, :], xt[:, k, :], pt1_sb[:])
                        nc.tensor.matmul(
                            bb_ps[:, gk % SUB, :], xt[:, k, :], pt2_sb[:]
                        )
                        if k % sstep == sstep - 1:
                            s_sb = mpool.tile([W, sstep, H], f16)
                            nc.scalar.copy(s_sb[:], s_ps[:])
                            s_tiles[k // sstep] = s_sb
                            # run stage 2 for every full ostep chunk now covered
                            flush_stage2((k + 1) - ((k + 1) % ostep))
                        if gk % SUB == SUB - 1:
                            si = gk // SUB  # global sub index
                            bb_sb = bbpool.tile([W, SUB * nbot], f16)
                            nc.scalar.copy(
                                bb_sb[:].rearrange("p (n r) -> p n r", n=SUB),
                                bb_ps[:],
                            )
                            ob_ps = pso.tile([SUB * nbot, NEW_W], f32)
                            nc.tensor.matmul(ob_ps[:], bb_sb[:], q_sb[:])
                            if si % obat == 0:
                                ob_acc = obapool.tile(
                                    [SUB * nbot, obat, NEW_W], f16, tag="oba"
                                )
                            cp = nc.scalar.copy if ob_act else nc.vector.tensor_copy
                            cp(ob_acc[:, si % obat, :], ob_ps[:])
                            if (si + 1) % obat == 0 and mode != "compute":
                                nc.sync.dma_start(
                                    obot[(si % (nslice // SUB)) // obat], ob_acc[:]
                                )
                    flush_stage2(group)
                    if mode != "compute":
                        nc.sync.dma_start(otop[:, sl, :], ot[:])

    nc.compile()
    return nc


_CACHE = {}

# Best HW-measured config: 2-slices-per-PSUM-bank compute with wide
# PSUM->SBUF copies, 32-slice DMA groups, everything fp32 (rel err ~4e-7).
CFG = {"v3": True}
MAP_KW = {"v3": True}


def _get_nc():
    if "nc" not in _CACHE:
        builder = _build_nc_v3 if CFG.get("v3") else _build_nc
        _CACHE["nc"] = builder(cfg=CFG)
    return _CACHE["nc"]


def make_in_maps(x, rate_weights, bf16x2=False, xf16=False, v3=False):
    p, q = _compute_pq(rate_weights)
    pt = np.ascontiguousarray(p.T)  # [128, 133]
    q = np.ascontiguousarray(q)
    xs = np.asarray(x, np.float32).reshape(N_CORES, NSLICE, H, W)
    # per-core permute to [H, NSLICE, W] so device DMA runs are contiguous
    shards = np.ascontiguousarray(xs.transpose(0, 2, 1, 3))
    if v3:
        shards = shards.astype(np.float16)
        pt1 = np.ascontiguousarray(pt[:, :H]).astype(np.float16)
        pt2 = np.ascontiguousarray(pt[:, H:]).astype(np.float16)
        q16 = q.astype(np.float16)
        return [
            {"x": shards[c], "pt1": pt1, "pt2": pt2, "q": q16}
            for c in range(N_CORES)
        ]
    if bf16x2:
        import ml_dtypes

        bf = ml_dtypes.bfloat16
        xh = shards.astype(bf)
        xl = (shards - xh.astype(np.float32)).astype(bf)
        pth = pt.astype(bf)
        ptl = (pt - pth.astype(np.float32)).astype(bf)
        return [
            {"xh": xh[c], "xl": xl[c], "pth": pth, "ptl": ptl, "q": q}
            for c in range(N_CORES)
        ]
    if xf16:
        shards = shards.astype(np.float16)
        pt = pt.astype(np.float16)
    return [{"x": shards[c], "pt": pt, "q": q} for c in range(N_CORES)]


def run(x, rate_weights, trace=False):
    """Returns (full_output, BassKernelResults)."""
    from concourse import bass_utils

    in_maps = make_in_maps(x, rate_weights, **MAP_KW)
    nc = _get_nc()
    res = bass_utils.run_bass_kernel_spmd(
        nc, in_maps, core_ids=list(range(N_CORES)), trace=trace
    )
    out = np.empty((B * C, NEW_H, NEW_W), np.float32)
    nbot = NEW_H - H
    for c in range(N_CORES):
        r = res.results[c]
        lo, hi = c * NSLICE, (c + 1) * NSLICE
        out[lo:hi, :H, :] = r["otop"].transpose(1, 0, 2)
        if CFG.get("v3"):
            ob = r["obot"]  # [nsg, 16*nbot, obat, NEW_W]
            nsg, _, obat, _ = ob.shape
            ob = ob.reshape(nsg, 16, nbot, obat, NEW_W).transpose(0, 3, 1, 2, 4)
            out[lo:hi, H:, :] = ob.reshape(NSLICE, nbot, NEW_W)
        else:
            out[lo:hi, H:, :] = r["obot"]
    return out.reshape(B, C, NEW_H, NEW_W), res


def kernel(x, rate_weights):
    out, _ = run(x, rate_weights)
    return out

